# revision 12
# baseline (speedup 1.0000x reference)
"""GAT (2-layer graph attention) Trainium2 Bass kernel, 8-core SPMD.

Sharding: data-parallel over batch (2) x row-blocks (4) -> 8 cores.
Core c handles batch b=c//4, output rows R=[512*(c%4), 512*(c%4+1)).

Key algebra: with z = s_src[i]+s_tgt[j], the GAT edge weight
exp(leaky_relu(z, 0.2)) = max(exp(z), exp(0.2 z)).  Softmax rows are
invariant to a per-row scale, so dividing row i by exp(s_src[i]) gives
unnormalized weights F[j,i] = adj[i,j] * D[j] * max(W[j], g[i]) with
  W[j] = exp(0.8 s_tgt[j]),  D[j] = exp(0.2 s_tgt[j]),  g[i] = exp(-0.8 s_src[i])
-- no per-element transcendentals.  D[j] is folded into the stationary
matmul operand (proj rows, plus a D-valued ones-column so the softmax
denominator falls out as matmul row 64).  The n x n inner work is one
fused DVE op per 128x512 tile: (g_bcast max W[j]) * adjT[j,i], feeding
the TensorE numerator matmul in bf16.

Layer-1 outputs are exchanged within each batch group of 4 cores via a
single AllGather of (proj2^T | s_tgt2) so layer 2 stays row-local.
"""

import os
import sys

for _p in ("/opt/trn_rl_repo", "/root/.axon_site/_ro/trn_rl_repo"):
    if os.path.isdir(_p) and _p not in sys.path:
        sys.path.insert(0, _p)

import numpy as np

import concourse.bass as bass
import concourse.bacc as bacc
import concourse.mybir as mybir
from concourse import tile
from concourse.bass_utils import run_bass_kernel_spmd

F32 = mybir.dt.float32
BF16 = mybir.dt.bfloat16
AF = mybir.ActivationFunctionType
ALU = mybir.AluOpType

BS, N, FIN = 2, 2048, 128
H1, F1 = 8, 64
RB = 512          # row block per core
NJT = N // 128    # 16 j-tiles
NIT = RB // 128   # 4 i-tiles in the row block
NCORES = 8
GROUPS = [[0, 1, 2, 3], [4, 5, 6, 7]]


def build_nc():
    nc = bacc.Bacc("TRN2", target_bir_lowering=False, debug=False,
                   num_devices=NCORES)

    # ---- per-core DRAM I/O ----
    d_x = nc.declare_dram_parameter("xb", [N, FIN], F32, isOutput=False)
    d_xr = nc.declare_dram_parameter("xbr", [RB, FIN], F32, isOutput=False)
    d_adj = nc.declare_dram_parameter("adjr", [RB, N], F32, isOutput=False)
    d_w1 = nc.declare_dram_parameter("w1", [H1 * F1, FIN], F32, isOutput=False)
    d_ws1 = nc.declare_dram_parameter("wskip1", [H1 * F1, FIN], F32, isOutput=False)
    d_as1 = nc.declare_dram_parameter("asrc1", [H1, F1], F32, isOutput=False)
    d_at1 = nc.declare_dram_parameter("atgt1", [H1, F1], F32, isOutput=False)
    d_b1 = nc.declare_dram_parameter("b1", [H1 * F1], F32, isOutput=False)
    d_w2 = nc.declare_dram_parameter("w2", [F1, H1 * F1], F32, isOutput=False)
    d_ws2 = nc.declare_dram_parameter("wskip2", [F1, H1 * F1], F32, isOutput=False)
    d_as2 = nc.declare_dram_parameter("asrc2", [1, F1], F32, isOutput=False)
    d_at2 = nc.declare_dram_parameter("atgt2", [1, F1], F32, isOutput=False)
    d_b2 = nc.declare_dram_parameter("b2", [F1], F32, isOutput=False)
    # output: transposed row-block out^T [64, 512] (host transposes back)
    d_out = nc.declare_dram_parameter("outT", [F1, RB], F32, isOutput=True)

    with tile.TileContext(nc) as tc:
        with (
            tc.tile_pool(name="persist", bufs=1) as P,
            tc.tile_pool(name="work", bufs=3) as WK,
            tc.tile_pool(name="gbp", bufs=2) as GB,
            tc.tile_pool(name="ps", bufs=4, space="PSUM") as PS,
            tc.tile_pool(name="psnum", bufs=2, space="PSUM") as PSN,
            tc.tile_pool(name="dram", bufs=1, space="DRAM") as DR,
        ):
            # ============ loads (transposes via strided DMA / cast-DMA) =====
            xT = P.tile([128, N], F32, tag="xT")
            nc.sync.dma_start(xT[:], d_x.rearrange("j c -> c j"))
            xTb = P.tile([128, N], BF16, tag="xTb")
            nc.vector.tensor_copy(xTb[:], xT[:])
            xrTf = P.tile([128, RB], F32, tag="xrTf")
            nc.sync.dma_start(xrTf[:], d_xr.rearrange("j c -> c j"))
            xrTb = P.tile([128, RB], BF16, tag="xrTb")
            nc.vector.tensor_copy(xrTb[:], xrTf[:])

            w1Tf = P.tile([128, H1 * F1], F32, tag="w1Tf")
            nc.sync.dma_start(w1Tf[:], d_w1.rearrange("f c -> c f"))
            w1Tb = P.tile([128, H1 * F1], BF16, tag="w1Tb")
            nc.vector.tensor_copy(w1Tb[:], w1Tf[:])
            ws1Tf = P.tile([128, H1 * F1], F32, tag="ws1Tf")
            nc.sync.dma_start(ws1Tf[:], d_ws1.rearrange("f c -> c f"))
            ws1Tb = P.tile([128, H1 * F1], BF16, tag="ws1Tb")
            nc.vector.tensor_copy(ws1Tb[:], ws1Tf[:])
            w2Tf = P.tile([128, 4, F1], F32, tag="w2Tf")
            ws2Tf = P.tile([128, 4, F1], F32, tag="ws2Tf")
            for kt in range(4):
                nc.sync.dma_start(
                    w2Tf[:, kt, :],
                    d_w2[:, kt * 128:(kt + 1) * 128].rearrange("f p -> p f"))
                nc.sync.dma_start(
                    ws2Tf[:, kt, :],
                    d_ws2[:, kt * 128:(kt + 1) * 128].rearrange("f p -> p f"))
            w2Tb = P.tile([128, 4, F1], BF16, tag="w2Tb")
            nc.vector.tensor_copy(w2Tb[:], w2Tf[:])
            ws2Tb = P.tile([128, 4, F1], BF16, tag="ws2Tb")
            nc.vector.tensor_copy(ws2Tb[:], ws2Tf[:])

            w1n = P.tile([128, 4, FIN], F32, tag="w1n")
            nc.sync.dma_start(w1n[:], d_w1.rearrange("(k p) c -> p k c", p=128))
            w2n = P.tile([F1, H1 * F1], F32, tag="w2n")
            nc.sync.dma_start(w2n[:], d_w2[:, :])

            b1sb = P.tile([128, 4], F32, tag="b1sb")
            nc.sync.dma_start(b1sb[:], d_b1.rearrange("(k p) -> p k", p=128))
            b2sb = P.tile([F1, 1], F32, tag="b2sb")
            nc.sync.dma_start(b2sb[:], d_b2.ap().rearrange("(f o) -> f o", o=1))

            a2p = P.tile([F1, 2], F32, tag="a2p")
            nc.sync.dma_start(a2p[:, 0:1], d_as2.rearrange("o f -> f o"))
            nc.sync.dma_start(a2p[:, 1:2], d_at2.rearrange("o f -> f o"))

            # A1emb [512(=128x4), 16]: col h = a_src1[h] at rows h*64..h*64+63,
            # col 8+h = a_tgt1[h]
            a1e = P.tile([128, 4, 16], F32, tag="a1e")
            nc.vector.memset(a1e[:], 0.0)
            for h in range(H1):
                kt, prow = (h * F1) // 128, (h * F1) % 128
                nc.sync.dma_start(a1e[prow:prow + F1, kt, h:h + 1],
                                  d_as1[h:h + 1, :].rearrange("o f -> f o"))
                nc.sync.dma_start(a1e[prow:prow + F1, kt, 8 + h:9 + h],
                                  d_at1[h:h + 1, :].rearrange("o f -> f o"))

            ones1b = P.tile([1, 128], BF16, tag="ones1b")
            nc.vector.memset(ones1b[:], 1.0)

            # ============ adjacency: cast-DMA + XBAR transpose ==============
            adjRb = P.tile([128, 4, N], BF16, tag="adjRb")
            for it in range(NIT):
                nc.gpsimd.dma_start(adjRb[:, it, :],
                                    d_adj[it * 128:(it + 1) * 128, :])
            adjT = P.tile([128, NJT, RB], BF16, tag="adjT")
            for jt in range(NJT):
                for it in range(NIT):
                    nc.sync.dma_start(
                        adjT[:, jt, it * 128:(it + 1) * 128],
                        adjRb[:, it, jt * 128:(jt + 1) * 128],
                        transpose=True)

            # ============ small exact fp32 matmuls ==========================
            # w1tilde [c=128, 16] = sum_F W1[F,c] * A1emb[F,t]
            ps_wt = PS.tile([128, 512], F32, tag="ps")
            for kt in range(4):
                nc.tensor.matmul(ps_wt[0:128, 0:16], w1n[:, kt, :], a1e[:, kt, :],
                                 start=(kt == 0), stop=(kt == 3))
            w1t = P.tile([128, 16], F32, tag="w1t")
            nc.vector.tensor_copy(w1t[:], ps_wt[0:128, 0:16])

            # S1T [j(128 x 16 chunks), 16] = x @ w1tilde
            ps_s1t = PS.tile([128, 512], F32, tag="ps")
            for jc in range(NJT):
                nc.tensor.matmul(ps_s1t[0:128, jc * 16:(jc + 1) * 16],
                                 xT[:, jc * 128:(jc + 1) * 128], w1t[:])
            s1T = P.tile([128, NJT * 16], F32, tag="s1T")
            nc.vector.tensor_copy(s1T[:], ps_s1t[0:128, 0:NJT * 16])
            Wv = P.tile([128, NJT * 16], F32, tag="Wv")
            nc.scalar.activation(Wv[:], s1T[:], AF.Exp, scale=0.8)
            Dv = P.tile([128, NJT * 16], F32, tag="Dv")
            nc.scalar.activation(Dv[:], s1T[:], AF.Exp, scale=0.2)

            # s_src for our rows: [16, 512];  g = exp(-0.8 s_src) in bf16
            ps_s1r = PS.tile([128, 512], F32, tag="ps")
            nc.tensor.matmul(ps_s1r[0:16, 0:RB], w1t[:], xrTf[:])
            g1b = P.tile([16, RB], BF16, tag="g1b")
            nc.scalar.activation(g1b[:], ps_s1r[0:16, 0:RB], AF.Exp, scale=-0.8)

            # ============ layer-1 skip:  (x_R @ Wskip1^T)^T  ================
            skipTb = P.tile([128, 4, RB], BF16, tag="skipTb")
            for pr in range(4):
                ps_sk = PS.tile([128, 512], F32, tag="ps")
                nc.tensor.matmul(ps_sk[0:128, 0:RB],
                                 ws1Tb[:, pr * 128:(pr + 1) * 128], xrTb[:])
                nc.scalar.activation(skipTb[:, pr, :], ps_sk[0:128, 0:RB], AF.Copy)

            # ============ proj1 (+ ones col, + D fold) ======================
            p1e = P.tile([128, NJT, 8 * 66], BF16, tag="p1e")
            for jt in range(NJT):
                ps_p = PS.tile([128, 512], F32, tag="ps")
                nc.tensor.matmul(ps_p[0:128, 0:512],
                                 xTb[:, jt * 128:(jt + 1) * 128], w1Tb[:])
                dst = p1e[:, jt, :].rearrange("p (h q) -> p h q", q=66)
                src = ps_p[0:128, 0:512].rearrange("p (h q) -> p h q", q=64)
                if jt % 2 == 0:
                    nc.vector.tensor_copy(dst[:, :, 0:64], src)
                else:
                    nc.scalar.activation(dst[:, :, 0:64], src, AF.Copy)
                nc.vector.memset(dst[:, :, 64:65], 1.0)
                for h in range(H1):
                    nc.vector.tensor_scalar(
                        p1e[:, jt, h * 66:h * 66 + 65],
                        p1e[:, jt, h * 66:h * 66 + 65],
                        Dv[:, jt * 16 + 8 + h:jt * 16 + 9 + h], None, ALU.mult)

            # ============ layer-1 head loop =================================
            numb = P.tile([128, 4, RB], BF16, tag="numb")  # numerators, 2 heads/tile
            recbs = []
            for h in range(H1):
                # stage this head's g row at partition 0 (engine APs need
                # base partition 0/32/64; DMA has no such restriction)
                grow = GB.tile([1, RB], BF16, tag="grow")
                nc.sync.dma_start(grow[:], g1b[h:h + 1, :])
                ps_g = PS.tile([128, 512], F32, tag="ps")
                nc.tensor.matmul(ps_g[0:128, 0:RB], ones1b[:], grow[:])
                gbh = GB.tile([128, RB], BF16, tag="gb")
                nc.vector.tensor_copy(gbh[:], ps_g[0:128, 0:RB])

                numT = PSN.tile([65, 512], F32, tag="numT")
                for jt in range(NJT):
                    Ft = WK.tile([128, RB], BF16, tag="F")
                    nc.vector.scalar_tensor_tensor(
                        Ft[:], gbh[:], Wv[:, jt * 16 + 8 + h:jt * 16 + 9 + h],
                        adjT[:, jt, :], ALU.max, ALU.mult)
                    nc.tensor.matmul(numT[0:65, 0:RB],
                                     p1e[:, jt, h * 66:h * 66 + 65], Ft[:],
                                     start=(jt == 0), stop=(jt == NJT - 1))
                den_h = P.tile([1, RB], F32, tag=f"den{h}")
                nc.scalar.activation(den_h[:], numT[64:65, 0:RB], AF.Copy)
                rec_h = P.tile([1, RB], F32, tag=f"rec{h}")
                nc.vector.reciprocal_approx_fast(rec_h[:], den_h[:])
                recb_h = P.tile([1, RB], BF16, tag=f"recb{h}")
                nc.vector.tensor_copy(recb_h[:], rec_h[:])
                recbs.append(recb_h)
                nc.scalar.activation(numb[(h % 2) * 64:(h % 2) * 64 + 64, h // 2, :],
                                     numT[0:64, 0:RB], AF.Copy)

            # h_out^T = elu(num/den + skip + b1), kept bf16
            houtb = P.tile([128, 4, RB], BF16, tag="houtb")
            for pr in range(4):
                rdb = GB.tile([128, RB], BF16, tag="rdb")
                for hh in range(2):
                    ps_r = PS.tile([128, 512], F32, tag="ps")
                    nc.tensor.matmul(ps_r[0:64, 0:RB], ones1b[0:1, 0:64],
                                     recbs[2 * pr + hh][:])
                    nc.vector.tensor_copy(rdb[hh * 64:hh * 64 + 64, :],
                                          ps_r[0:64, 0:RB])
                hpre = WK.tile([128, RB], BF16, tag="hpre")
                nc.vector.tensor_mul(hpre[:], numb[:, pr, :], rdb[:])
                u = WK.tile([128, RB], BF16, tag="u")
                nc.vector.scalar_tensor_tensor(
                    u[:], hpre[:], b1sb[:, pr:pr + 1], skipTb[:, pr, :],
                    ALU.add, ALU.add)
                m0 = WK.tile([128, RB], BF16, tag="m0")
                nc.vector.tensor_scalar(m0[:], u[:], 0.0, None, ALU.min)
                e = WK.tile([128, RB], BF16, tag="e")
                nc.scalar.activation(e[:], m0[:], AF.Exp)
                nc.vector.scalar_tensor_tensor(
                    houtb[:, pr, :], e[:], -1.0, u[:], ALU.add, ALU.max)

            # ============ layer-2 local pieces ==============================
            # w2tilde [c2(128x4), 2]
            ps_w2 = PS.tile([128, 512], F32, tag="ps")
            for kt in range(4):
                nc.tensor.matmul(ps_w2[0:128, kt * 2:kt * 2 + 2],
                                 w2n[:, kt * 128:(kt + 1) * 128], a2p[:],
                                 start=True, stop=True)
            w2tb = P.tile([128, 8], BF16, tag="w2tb")
            nc.vector.tensor_copy(w2tb[:], ps_w2[0:128, 0:8])

            # S2: s_src2 -> psum row 0, s_tgt2 -> psum row 32
            ps_s2 = PS.tile([128, 512], F32, tag="ps")
            for kt in range(4):
                nc.tensor.matmul(ps_s2[0:1, 0:RB], w2tb[:, kt * 2:kt * 2 + 1],
                                 houtb[:, kt, :], start=(kt == 0), stop=(kt == 3))
            for kt in range(4):
                nc.tensor.matmul(ps_s2[32:33, 0:RB], w2tb[:, kt * 2 + 1:kt * 2 + 2],
                                 houtb[:, kt, :], start=(kt == 0), stop=(kt == 3))
            g2row = P.tile([1, RB], BF16, tag="g2row")
            nc.scalar.activation(g2row[:], ps_s2[0:1, 0:RB], AF.Exp, scale=-0.8)
            stg2 = P.tile([1, RB], F32, tag="stg2")
            nc.scalar.activation(stg2[:], ps_s2[32:33, 0:RB], AF.Copy)

            # proj2^T local [64, 512]
            ps_p2 = PS.tile([128, 512], F32, tag="ps")
            for kt in range(4):
                nc.tensor.matmul(ps_p2[0:64, 0:RB], w2Tb[:, kt, :],
                                 houtb[:, kt, :], start=(kt == 0), stop=(kt == 3))
            p2T = P.tile([F1, RB], F32, tag="p2T")
            nc.scalar.activation(p2T[:], ps_p2[0:64, 0:RB], AF.Copy)

            # ============ AllGather within batch group ======================
            gin = DR.tile([F1 + 1, RB], F32)
            nc.sync.dma_start(gin[0:F1, :], p2T[:])
            nc.sync.dma_start(gin[F1:F1 + 1, :], stg2[:])
            gout = DR.tile([4, F1 + 1, RB], F32)
            nc.gpsimd.collective_compute(
                "AllGather", ALU.bypass, replica_groups=GROUPS,
                ins=[gin.opt()], outs=[gout.opt()])

            # ============ layer-2 attention =================================
            stage = P.tile([128, 4, 4, F1 + 1], F32, tag="stage")
            for c in range(4):
                for sb in range(4):
                    nc.sync.dma_start(
                        stage[:, c, sb, :],
                        gout[c, :, sb * 128:(sb + 1) * 128]
                        .rearrange("f p -> p f"))
            p2e = P.tile([128, 4, 4, F1 + 1], BF16, tag="p2e")
            nc.vector.tensor_copy(p2e[:], stage[:])
            # col 64 carried s_tgt2 through the gather; the denominator
            # column must be 1 (becomes D2 after the fold below)
            nc.vector.memset(p2e[:, :, :, F1:F1 + 1], 1.0)
            st2T = P.tile([128, 4, 4], F32, tag="st2T")
            for c in range(4):
                nc.sync.dma_start(
                    st2T[:, c, :],
                    gout[c, F1, :].rearrange("(s p) -> p s", p=128))
            W2v = P.tile([128, 4, 4], F32, tag="W2v")
            nc.scalar.activation(W2v[:], st2T[:], AF.Exp, scale=0.8)
            D2v = P.tile([128, 4, 4], F32, tag="D2v")
            nc.scalar.activation(D2v[:], st2T[:], AF.Exp, scale=0.2)
            for jt in range(NJT):
                nc.vector.tensor_scalar(p2e[:, jt // 4, jt % 4, :],
                                        p2e[:, jt // 4, jt % 4, :],
                                        D2v[:, jt // 4, jt % 4:jt % 4 + 1],
                                        None, ALU.mult)

            ps_g2 = PS.tile([128, 512], F32, tag="ps")
            nc.tensor.matmul(ps_g2[0:128, 0:RB], ones1b[:], g2row[:])
            g2bc = GB.tile([128, RB], BF16, tag="gb")
            nc.vector.tensor_copy(g2bc[:], ps_g2[0:128, 0:RB])

            numT2 = PSN.tile([65, 512], F32, tag="numT")
            for jt in range(NJT):
                F2 = WK.tile([128, RB], BF16, tag="F")
                nc.vector.scalar_tensor_tensor(
                    F2[:], g2bc[:], W2v[:, jt // 4, jt % 4:jt % 4 + 1],
                    adjT[:, jt, :], ALU.max, ALU.mult)
                nc.tensor.matmul(numT2[0:65, 0:RB], p2e[:, jt // 4, jt % 4, :],
                                 F2[:], start=(jt == 0), stop=(jt == NJT - 1))

            den2 = P.tile([1, RB], F32, tag="den2")
            nc.scalar.activation(den2[:], numT2[64:65, 0:RB], AF.Copy)
            rec2 = P.tile([1, RB], F32, tag="rec2")
            nc.vector.reciprocal_approx_fast(rec2[:], den2[:])
            rec2b = P.tile([1, RB], BF16, tag="rec2b")
            nc.vector.tensor_copy(rec2b[:], rec2[:])
            ps_r2 = PS.tile([128, 512], F32, tag="ps")
            nc.tensor.matmul(ps_r2[0:64, 0:RB], ones1b[0:1, 0:64], rec2b[:])
            rdb2 = GB.tile([128, RB], BF16, tag="rdb")
            nc.vector.tensor_copy(rdb2[0:64, :], ps_r2[0:64, 0:RB])

            ps_sk2 = PS.tile([128, 512], F32, tag="ps")
            for kt in range(4):
                nc.tensor.matmul(ps_sk2[0:64, 0:RB], ws2Tb[:, kt, :],
                                 houtb[:, kt, :], start=(kt == 0), stop=(kt == 3))

            t2 = WK.tile([F1, RB], F32, tag="t2")
            nc.vector.tensor_mul(t2[:], numT2[0:64, 0:RB], rdb2[0:64, :])
            o2 = WK.tile([F1, RB], F32, tag="o2")
            nc.vector.scalar_tensor_tensor(
                o2[:], t2[:], b2sb[:], ps_sk2[0:64, 0:RB], ALU.add, ALU.add)
            nc.sync.dma_start(d_out[:, :], o2[:])

    nc.compile()
    return nc


_NC_CACHE = None


def _get_nc():
    global _NC_CACHE
    if _NC_CACHE is None:
        _NC_CACHE = build_nc()
    return _NC_CACHE


def kernel(x, adj, W1, a_src1, a_tgt1, Wskip1, b1, W2, a_src2, a_tgt2,
           Wskip2, b2):
    x = np.asarray(x, np.float32)
    adj = np.asarray(adj, np.float32)
    nc = _get_nc()
    in_maps = []
    for c in range(NCORES):
        b, r = c // 4, c % 4
        sl = slice(r * RB, (r + 1) * RB)
        in_maps.append({
            "xb": x[b], "xbr": x[b][sl], "adjr": adj[b][sl],
            "w1": np.asarray(W1, np.float32),
            "wskip1": np.asarray(Wskip1, np.float32),
            "asrc1": np.asarray(a_src1, np.float32),
            "atgt1": np.asarray(a_tgt1, np.float32),
            "b1": np.asarray(b1, np.float32),
            "w2": np.asarray(W2, np.float32),
            "wskip2": np.asarray(Wskip2, np.float32),
            "asrc2": np.asarray(a_src2, np.float32),
            "atgt2": np.asarray(a_tgt2, np.float32),
            "b2": np.asarray(b2, np.float32),
        })
    res = run_bass_kernel_spmd(nc, in_maps, core_ids=list(range(NCORES)))
    out = np.empty((BS, N, F1), np.float32)
    for c in range(NCORES):
        b, r = c // 4, c % 4
        out[b, r * RB:(r + 1) * RB, :] = res.results[c]["outT"].T
    return out


# revision 32
# speedup vs baseline: 785.6746x; 785.6746x over previous
"""GAT (2-layer graph attention) Trainium2 Bass kernel, 8-core SPMD.

Sharding: data-parallel over batch (2) x row-blocks (4) -> 8 cores.
Core c handles batch b=c//4, output rows R=[512*(c%4), 512*(c%4+1)).

Key algebra: with z = s_src[i]+s_tgt[j], the GAT edge weight
exp(leaky_relu(z, 0.2)) = max(exp(z), exp(0.2 z)).  Softmax rows are
invariant to a per-row scale, so dividing row i by exp(s_src[i]) gives
unnormalized weights F[j,i] = adj[i,j] * D[j] * max(W[j], g[i]) with
  W[j] = exp(0.8 s_tgt[j]),  D[j] = exp(0.2 s_tgt[j]),  g[i] = exp(-0.8 s_src[i])
-- no per-element transcendentals.  D[j] is folded into the stationary
matmul operand (proj rows, plus a D-valued ones-column so the softmax
denominator falls out as matmul row 64).  The n x n inner work is one
fused DVE op per 128x512 tile: (g_bcast max W[j]) * adjT[j,i], feeding
the TensorE numerator matmul in bf16.

Layer-1 outputs are exchanged within each batch group of 4 cores via a
single AllGather of (proj2^T | s_tgt2) so layer 2 stays row-local.
"""

import os
import sys

for _p in ("/opt/trn_rl_repo", "/root/.axon_site/_ro/trn_rl_repo"):
    if os.path.isdir(_p) and _p not in sys.path:
        sys.path.insert(0, _p)

import numpy as np

import concourse.bass as bass
import concourse.bacc as bacc
import concourse.mybir as mybir
from concourse import tile
from concourse.bass_utils import run_bass_kernel_spmd

F32 = mybir.dt.float32
BF16 = mybir.dt.bfloat16
AF = mybir.ActivationFunctionType
ALU = mybir.AluOpType

BS, N, FIN = 2, 2048, 128
H1, F1 = 8, 64
RB = 512          # row block per core
NJT = N // 128    # 16 j-tiles
NIT = RB // 128   # 4 i-tiles in the row block
NCORES = 8
GROUPS = [[0, 1, 2, 3], [4, 5, 6, 7]]


def build_nc():
    nc = bacc.Bacc("TRN2", target_bir_lowering=False, debug=False,
                   num_devices=NCORES)

    # ---- per-core DRAM I/O ----
    d_x = nc.declare_dram_parameter("xb", [N, FIN], F32, isOutput=False)
    d_xr = nc.declare_dram_parameter("xbr", [RB, FIN], F32, isOutput=False)
    d_adj = nc.declare_dram_parameter("adjr", [RB, N], F32, isOutput=False)
    d_w1 = nc.declare_dram_parameter("w1", [H1 * F1, FIN], F32, isOutput=False)
    d_ws1 = nc.declare_dram_parameter("wskip1", [H1 * F1, FIN], F32, isOutput=False)
    d_as1 = nc.declare_dram_parameter("asrc1", [H1, F1], F32, isOutput=False)
    d_at1 = nc.declare_dram_parameter("atgt1", [H1, F1], F32, isOutput=False)
    d_b1 = nc.declare_dram_parameter("b1", [H1 * F1], F32, isOutput=False)
    d_w2 = nc.declare_dram_parameter("w2", [F1, H1 * F1], F32, isOutput=False)
    d_ws2 = nc.declare_dram_parameter("wskip2", [F1, H1 * F1], F32, isOutput=False)
    d_as2 = nc.declare_dram_parameter("asrc2", [1, F1], F32, isOutput=False)
    d_at2 = nc.declare_dram_parameter("atgt2", [1, F1], F32, isOutput=False)
    d_b2 = nc.declare_dram_parameter("b2", [F1], F32, isOutput=False)
    # output: transposed row-block out^T [64, 512] (host transposes back)
    d_out = nc.declare_dram_parameter("outT", [F1, RB], F32, isOutput=True)

    with tile.TileContext(nc) as tc:
        with (
            tc.tile_pool(name="persist", bufs=1) as P,
            tc.tile_pool(name="work", bufs=3) as WK,
            tc.tile_pool(name="gbp", bufs=2) as GB,
            tc.tile_pool(name="ps", bufs=4, space="PSUM") as PS,
            tc.tile_pool(name="psnum", bufs=2, space="PSUM") as PSN,
            tc.tile_pool(name="pst", bufs=2, space="PSUM") as PST,
            tc.tile_pool(name="dram", bufs=1, space="DRAM") as DR,
        ):
            # ============ loads (transposes via strided DMA) ================
            # spread across the two HWDGE queues (SP=sync, ACT=scalar)
            xT = P.tile([128, N], F32, tag="xT")
            nc.sync.dma_start(xT[:], d_x.rearrange("j c -> c j"))
            xTb = P.tile([128, N], BF16, tag="xTb")
            nc.vector.tensor_copy(xTb[:], xT[:])
            xrTf = P.tile([128, RB], F32, tag="xrTf")
            nc.scalar.dma_start(xrTf[:], d_xr.rearrange("j c -> c j"))
            xrTb = P.tile([128, RB], BF16, tag="xrTb")
            nc.vector.tensor_copy(xrTb[:], xrTf[:])

            ws1Tf = P.tile([128, H1 * F1], F32, tag="ws1Tf")
            nc.scalar.dma_start(ws1Tf[:], d_ws1.rearrange("f c -> c f"))
            ws1Tb = P.tile([128, H1 * F1], BF16, tag="ws1Tb")
            nc.vector.tensor_copy(ws1Tb[:], ws1Tf[:])

            w1n = P.tile([128, 4, FIN], F32, tag="w1n")
            nc.sync.dma_start(w1n[:], d_w1.rearrange("(k p) c -> p k c", p=128))
            w2n = P.tile([F1, H1 * F1], F32, tag="w2n")
            nc.scalar.dma_start(w2n[:], d_w2[:, :])
            ws2n = P.tile([F1, H1 * F1], F32, tag="ws2n")
            nc.scalar.dma_start(ws2n[:], d_ws2[:, :])

            b1f = P.tile([128, 4], F32, tag="b1f")
            nc.sync.dma_start(b1f[:], d_b1.rearrange("(k p) -> p k", p=128))
            b1sb = P.tile([128, 4], BF16, tag="b1sb")
            nc.vector.tensor_copy(b1sb[:], b1f[:])
            b2f = P.tile([F1, 1], F32, tag="b2f")
            nc.sync.dma_start(b2f[:], d_b2.ap().rearrange("(f o) -> f o", o=1))
            b2sb = P.tile([F1, 1], BF16, tag="b2sb")
            nc.vector.tensor_copy(b2sb[:], b2f[:])

            # a-vector tables, transposed on load: [128, 8] (col h = head h,
            # duplicated in both partition halves so matmul base matches W1)
            a1sT = P.tile([128, H1], F32, tag="a1sT")
            nc.sync.dma_start(a1sT[0:F1, :], d_as1.rearrange("h f -> f h"))
            nc.sync.dma_start(a1sT[F1:2 * F1, :], d_as1.rearrange("h f -> f h"))
            a1tT = P.tile([128, H1], F32, tag="a1tT")
            nc.sync.dma_start(a1tT[0:F1, :], d_at1.rearrange("h f -> f h"))
            nc.sync.dma_start(a1tT[F1:2 * F1, :], d_at1.rearrange("h f -> f h"))
            a2p = P.tile([F1, 2], F32, tag="a2p")
            nc.scalar.dma_start(a2p[:, 0:1], d_as2.rearrange("o f -> f o"))
            nc.scalar.dma_start(a2p[:, 1:2], d_at2.rearrange("o f -> f o"))

            ones1b = P.tile([1, 128], BF16, tag="ones1b")
            nc.vector.memset(ones1b[:], 1.0)

            # identity (bf16) for PE transposes
            onesq = P.tile([128, 128], BF16, tag="onesq")
            nc.vector.memset(onesq[:], 1.0)
            ident = P.tile([128, 128], BF16, tag="ident")
            nc.gpsimd.affine_select(ident[:], onesq[:], [[-1, 128]],
                                    ALU.is_equal, 0.0, base=0,
                                    channel_multiplier=1)
            identf = P.tile([128, 128], F32, tag="identf")
            onesqf = P.tile([128, 128], F32, tag="onesqf")
            nc.vector.memset(onesqf[:], 1.0)
            nc.gpsimd.affine_select(identf[:], onesqf[:], [[-1, 128]],
                                    ALU.is_equal, 0.0, base=0,
                                    channel_multiplier=1)
            # W1^T / W2^T / Wskip2^T (bf16) via PE transposes of natural tiles
            w1Tb = P.tile([128, H1 * F1], BF16, tag="w1Tb")
            for kt in range(4):
                ps_w = PS.tile([128, 512], F32, tag="ps")
                nc.tensor.transpose(ps_w[0:128, 0:128], w1n[:, kt, :], identf[:])
                nc.vector.tensor_copy(w1Tb[:, kt * 128:(kt + 1) * 128],
                                      ps_w[0:128, 0:128])
            w2Tb = P.tile([128, 4, F1], BF16, tag="w2Tb")
            ws2Tb = P.tile([128, 4, F1], BF16, tag="ws2Tb")
            for kt in range(4):
                ps_w = PS.tile([128, 512], F32, tag="ps")
                nc.tensor.transpose(ps_w[0:128, 0:64],
                                    w2n[:, kt * 128:(kt + 1) * 128],
                                    identf[0:64, 0:64])
                nc.vector.tensor_copy(w2Tb[:, kt, :], ps_w[0:128, 0:64])
            for kt in range(4):
                ps_w = PS.tile([128, 512], F32, tag="ps")
                nc.tensor.transpose(ps_w[0:128, 0:64],
                                    ws2n[:, kt * 128:(kt + 1) * 128],
                                    identf[0:64, 0:64])
                nc.vector.tensor_copy(ws2Tb[:, kt, :], ps_w[0:128, 0:64])

            # selector tiles sel_h [16, 128] (row h ones) for g broadcasts
            ones16 = P.tile([16, 128], BF16, tag="ones16")
            nc.vector.memset(ones16[:], 1.0)
            sel = P.tile([16, H1 * 128], BF16, tag="sel")
            for h in range(H1):
                nc.gpsimd.affine_select(sel[:, h * 128:(h + 1) * 128],
                                        ones16[:], [[0, 128]], ALU.is_equal,
                                        0.0, base=-h, channel_multiplier=1)

            # ============ adjacency: cast-DMA + PE transpose ================
            adjRb = P.tile([128, 4, N], BF16, tag="adjRb")
            adjRf = P.tile([128, 2, N], F32, tag="adjRf")
            for it in range(2):
                nc.gpsimd.dma_start(adjRb[:, it, :],
                                    d_adj[it * 128:(it + 1) * 128, :])
            for it in range(2, 4):
                nc.sync.dma_start(adjRf[:, it - 2, :],
                                  d_adj[it * 128:(it + 1) * 128, :])
                nc.vector.tensor_copy(adjRb[:, it, :], adjRf[:, it - 2, :])
            adjT = P.tile([128, NJT, RB], BF16, tag="adjT")
            for jt in range(NJT):
                ps_t = PST.tile([128, 512], BF16, tag="pst")
                for it in range(NIT):
                    nc.tensor.transpose(ps_t[:, it * 128:(it + 1) * 128],
                                        adjRb[:, it, jt * 128:(jt + 1) * 128],
                                        ident[:])
                if jt % 2 == 0:
                    nc.vector.tensor_copy(adjT[:, jt, :], ps_t[:])
                else:
                    nc.scalar.activation(adjT[:, jt, :], ps_t[:], AF.Copy)

            # ============ small exact fp32 matmuls ==========================
            # w1tilde [c=128, 16]: col h = W1_h^T a_src1[h], col 8+h tgt
            ps_wt = PS.tile([128, 512], F32, tag="ps")
            for h in range(H1):
                kt, pr = (h * F1) // 128, (h * F1) % 128
                w1slc = w1n[pr:pr + F1, kt, :]
                nc.tensor.matmul(ps_wt[0:128, h:h + 1], w1slc,
                                 a1sT[pr:pr + F1, h:h + 1])
                nc.tensor.matmul(ps_wt[0:128, 8 + h:9 + h], w1slc,
                                 a1tT[pr:pr + F1, h:h + 1])
            w1t = P.tile([128, 16], F32, tag="w1t")
            nc.vector.tensor_copy(w1t[:], ps_wt[0:128, 0:16])

            # S1T [j(128 x 16 chunks), 16] = x @ w1tilde
            ps_s1t = PS.tile([128, 512], F32, tag="ps")
            for jc in range(NJT):
                nc.tensor.matmul(ps_s1t[0:128, jc * 16:(jc + 1) * 16],
                                 xT[:, jc * 128:(jc + 1) * 128], w1t[:])
            s1T = P.tile([128, NJT * 16], F32, tag="s1T")
            nc.vector.tensor_copy(s1T[:], ps_s1t[0:128, 0:NJT * 16])
            Wvf = P.tile([128, NJT * 16], F32, tag="Wvf")
            nc.scalar.activation(Wvf[:], s1T[:], AF.Exp, scale=0.8)
            Dvf = P.tile([128, NJT * 16], F32, tag="Dvf")
            nc.scalar.activation(Dvf[:], s1T[:], AF.Exp, scale=0.2)

            # s_src rows for our block -> g (bf16) [16, 512]
            ps_s1r = PS.tile([128, 512], F32, tag="ps")
            nc.tensor.matmul(ps_s1r[0:16, 0:RB], w1t[:], xrTf[:])
            g1b = P.tile([16, RB], BF16, tag="g1b")
            nc.scalar.activation(g1b[:], ps_s1r[0:16, 0:RB], AF.Exp, scale=-0.8)

            # ============ layer-1 skip:  (x_R @ Wskip1^T)^T  ================
            skipTb = P.tile([128, 4, RB], BF16, tag="skipTb")
            for pr in range(4):
                ps_sk = PS.tile([128, 512], F32, tag="ps")
                nc.tensor.matmul(ps_sk[0:128, 0:RB],
                                 ws1Tb[:, pr * 128:(pr + 1) * 128], xrTb[:])
                nc.scalar.activation(skipTb[:, pr, :], ps_sk[0:128, 0:RB], AF.Copy)

            # ============ proj1 (+ ones col, + D fold on gpsimd) ============
            p1e = P.tile([128, NJT, 8 * 66], BF16, tag="p1e")
            for jt in range(NJT):
                ps_p = PS.tile([128, 512], F32, tag="ps")
                nc.tensor.matmul(ps_p[0:128, 0:512],
                                 xTb[:, jt * 128:(jt + 1) * 128], w1Tb[:])
                dst = p1e[:, jt, :].rearrange("p (h q) -> p h q", q=66)
                src = ps_p[0:128, 0:512].rearrange("p (h q) -> p h q", q=64)
                if jt % 2 == 0:
                    nc.vector.tensor_copy(dst[:, :, 0:64], src)
                else:
                    nc.scalar.activation(dst[:, :, 0:64], src, AF.Copy)
                nc.vector.memset(dst[:, :, 64:65], 1.0)

            # ============ layer-1 head loop =================================
            numb = P.tile([128, 4, RB], BF16, tag="numb")
            recbs = []
            for h in range(H1):
                ps_g = PS.tile([128, 512], F32, tag="ps")
                nc.tensor.matmul(ps_g[0:128, 0:RB],
                                 sel[:, h * 128:(h + 1) * 128], g1b[:])
                gbh = GB.tile([128, RB], BF16, tag="gb")
                nc.scalar.activation(gbh[:], ps_g[0:128, 0:RB], AF.Copy)

                numT = PSN.tile([65, 512], F32, tag="numT")
                for jt in range(NJT):
                    col = slice(jt * 16 + 8 + h, jt * 16 + 9 + h)
                    q = WK.tile([128, RB], BF16, tag="q")
                    Ft = WK.tile([128, RB], BF16, tag="F")
                    eng = nc.gpsimd if (jt % 3 == 2 or jt == 7) else nc.vector
                    eng.tensor_scalar(q[:], gbh[:], Wvf[:, col], Dvf[:, col],
                                      ALU.max, ALU.mult)
                    eng.tensor_tensor(Ft[:], q[:], adjT[:, jt, :], ALU.mult)
                    nc.tensor.matmul(numT[0:65, 0:RB],
                                     p1e[:, jt, h * 66:h * 66 + 65], Ft[:],
                                     start=(jt == 0), stop=(jt == NJT - 1))
                den_h = P.tile([1, RB], F32, tag=f"den{h}")
                nc.scalar.activation(den_h[:], numT[64:65, 0:RB], AF.Copy)
                rec_h = P.tile([1, RB], F32, tag=f"rec{h}")
                nc.vector.reciprocal_approx_fast(rec_h[:], den_h[:])
                recb_h = P.tile([1, RB], BF16, tag=f"recb{h}")
                nc.vector.tensor_copy(recb_h[:], rec_h[:])
                recbs.append(recb_h)
                nc.scalar.activation(numb[(h % 2) * 64:(h % 2) * 64 + 64, h // 2, :],
                                     numT[0:64, 0:RB], AF.Copy)

            # h_out^T = elu(num/den + skip + b1), kept bf16
            houtb = P.tile([128, 4, RB], BF16, tag="houtb")
            for pr in range(4):
                rdb = GB.tile([128, RB], BF16, tag="rdb")
                ps_r = PS.tile([128, 512], F32, tag="ps")
                nc.tensor.matmul(ps_r[0:64, 0:RB], ones1b[0:1, 0:64],
                                 recbs[2 * pr][:])
                nc.tensor.matmul(ps_r[64:128, 0:RB], ones1b[0:1, 0:64],
                                 recbs[2 * pr + 1][:])
                nc.scalar.activation(rdb[:], ps_r[0:128, 0:RB], AF.Copy)
                hpre = WK.tile([128, RB], BF16, tag="hpre")
                nc.vector.tensor_mul(hpre[:], numb[:, pr, :], rdb[:])
                u = WK.tile([128, RB], BF16, tag="u")
                nc.vector.scalar_tensor_tensor(
                    u[:], hpre[:], b1sb[:, pr:pr + 1], skipTb[:, pr, :],
                    ALU.add, ALU.add)
                m0 = WK.tile([128, RB], BF16, tag="m0")
                nc.vector.tensor_scalar(m0[:], u[:], 0.0, None, ALU.min)
                e = WK.tile([128, RB], BF16, tag="e")
                nc.scalar.activation(e[:], m0[:], AF.Exp)
                nc.vector.scalar_tensor_tensor(
                    houtb[:, pr, :], e[:], -1.0, u[:], ALU.add, ALU.max)

            # ============ layer-2 local pieces ==============================
            ps_w2 = PS.tile([128, 512], F32, tag="ps")
            for kt in range(4):
                nc.tensor.matmul(ps_w2[0:128, kt * 2:kt * 2 + 2],
                                 w2n[:, kt * 128:(kt + 1) * 128], a2p[:],
                                 start=True, stop=True)
            w2tb = P.tile([128, 8], BF16, tag="w2tb")
            nc.vector.tensor_copy(w2tb[:], ps_w2[0:128, 0:8])

            # S2: s_src2 -> psum row 0, s_tgt2 -> psum row 32
            ps_s2 = PS.tile([128, 512], F32, tag="ps")
            for kt in range(4):
                nc.tensor.matmul(ps_s2[0:1, 0:RB], w2tb[:, kt * 2:kt * 2 + 1],
                                 houtb[:, kt, :], start=(kt == 0), stop=(kt == 3))
            for kt in range(4):
                nc.tensor.matmul(ps_s2[32:33, 0:RB], w2tb[:, kt * 2 + 1:kt * 2 + 2],
                                 houtb[:, kt, :], start=(kt == 0), stop=(kt == 3))
            g2row = P.tile([1, RB], BF16, tag="g2row")
            nc.scalar.activation(g2row[:], ps_s2[0:1, 0:RB], AF.Exp, scale=-0.8)
            stg2 = P.tile([1, RB], F32, tag="stg2")
            nc.scalar.activation(stg2[:], ps_s2[32:33, 0:RB], AF.Copy)

            # proj2^T local [64, 512] in bf16 for the gather
            ps_p2 = PS.tile([128, 512], F32, tag="ps")
            for kt in range(4):
                nc.tensor.matmul(ps_p2[0:64, 0:RB], w2Tb[:, kt, :],
                                 houtb[:, kt, :], start=(kt == 0), stop=(kt == 3))
            p2Tb = P.tile([F1, RB], BF16, tag="p2Tb")
            nc.scalar.activation(p2Tb[:], ps_p2[0:64, 0:RB], AF.Copy)

            # ============ AllGather within batch group ======================
            # gin2 bf16 [4, 65, 128]: rows 0..63 = proj2^T slices, row 64 =
            # s_tgt2 (bf16 value -- 4e-3 total error verified in mock)
            gin = DR.tile([4, F1 + 1, 128], BF16)
            nc.sync.dma_start(
                gin[:, 0:F1, :].rearrange("s f p -> f s p"),
                p2Tb[:].rearrange("f (s p) -> f s p", p=128))
            stg2b = P.tile([1, RB], BF16, tag="stg2b")
            nc.vector.tensor_copy(stg2b[:], stg2[:])
            nc.sync.dma_start(
                gin[:, F1:F1 + 1, :].rearrange("s o p -> o s p"),
                stg2b[:].rearrange("o (s p) -> o s p", p=128))
            gout = DR.tile([4, 4, F1 + 1, 128], BF16)
            nc.gpsimd.collective_compute(
                "AllGather", ALU.bypass, replica_groups=GROUPS,
                ins=[gin.opt()], outs=[gout.opt()])

            # ============ layer-2 attention =================================
            p2e = P.tile([128, NJT, F1 + 1], BF16, tag="p2e")
            nc.sync.dma_start(
                p2e[:],
                gout.rearrange("c s f p -> p (c s) f"))
            # the denominator column must be 1
            nc.vector.memset(p2e[:, :, F1:F1 + 1], 1.0)
            st2Tb = P.tile([128, 4, 4], BF16, tag="st2Tb")
            nc.scalar.dma_start(
                st2Tb[:], gout[:, :, F1, :].rearrange("c s p -> p c s"))
            st2T = P.tile([128, 4, 4], F32, tag="st2T")
            nc.scalar.activation(st2T[:], st2Tb[:], AF.Copy)
            D2v = P.tile([128, 4, 4], F32, tag="D2v")
            nc.scalar.activation(D2v[:], st2T[:], AF.Exp, scale=0.2)

            ps_g2 = PS.tile([128, 512], F32, tag="ps")
            nc.tensor.matmul(ps_g2[0:128, 0:RB], ones1b[:], g2row[:])
            g2bc = GB.tile([128, RB], BF16, tag="gb")
            nc.vector.tensor_copy(g2bc[:], ps_g2[0:128, 0:RB])

            W2vf = P.tile([128, 4, 4], F32, tag="W2vf")
            nc.scalar.activation(W2vf[:], st2T[:], AF.Exp, scale=0.8)
            numT2 = PSN.tile([65, 512], F32, tag="numT")
            for jt in range(NJT):
                c4, s4 = jt // 4, jt % 4
                q2 = WK.tile([128, RB], BF16, tag="q")
                F2 = WK.tile([128, RB], BF16, tag="F")
                eng = nc.gpsimd if (jt % 3 == 2 or jt == 7) else nc.vector
                eng.tensor_scalar(q2[:], g2bc[:], W2vf[:, c4, s4:s4 + 1],
                                  D2v[:, c4, s4:s4 + 1], ALU.max, ALU.mult)
                eng.tensor_tensor(F2[:], q2[:], adjT[:, jt, :], ALU.mult)
                nc.tensor.matmul(numT2[0:65, 0:RB], p2e[:, jt, :],
                                 F2[:], start=(jt == 0), stop=(jt == NJT - 1))

            den2 = P.tile([1, RB], F32, tag="den2")
            nc.scalar.activation(den2[:], numT2[64:65, 0:RB], AF.Copy)
            rec2 = P.tile([1, RB], F32, tag="rec2")
            nc.vector.reciprocal_approx_fast(rec2[:], den2[:])
            rec2b = P.tile([1, RB], BF16, tag="rec2b")
            nc.vector.tensor_copy(rec2b[:], rec2[:])
            ps_r2 = PS.tile([128, 512], F32, tag="ps")
            nc.tensor.matmul(ps_r2[0:64, 0:RB], ones1b[0:1, 0:64], rec2b[:])
            rdb2 = GB.tile([128, RB], BF16, tag="rdb")
            nc.vector.tensor_copy(rdb2[0:64, :], ps_r2[0:64, 0:RB])

            ps_sk2 = PS.tile([128, 512], F32, tag="ps")
            for kt in range(4):
                nc.tensor.matmul(ps_sk2[0:64, 0:RB], ws2Tb[:, kt, :],
                                 houtb[:, kt, :], start=(kt == 0), stop=(kt == 3))

            t2 = WK.tile([F1, RB], F32, tag="t2")
            nc.vector.tensor_mul(t2[:], numT2[0:64, 0:RB], rdb2[0:64, :])
            o2 = WK.tile([F1, RB], F32, tag="o2")
            nc.vector.scalar_tensor_tensor(
                o2[:], t2[:], b2f[:], ps_sk2[0:64, 0:RB], ALU.add, ALU.add)
            nc.sync.dma_start(d_out[:, :], o2[:])

    nc.compile()
    return nc


_NC_CACHE = None


def _get_nc():
    global _NC_CACHE
    if _NC_CACHE is None:
        _NC_CACHE = build_nc()
    return _NC_CACHE


def kernel(x, adj, W1, a_src1, a_tgt1, Wskip1, b1, W2, a_src2, a_tgt2,
           Wskip2, b2):
    x = np.asarray(x, np.float32)
    adj = np.asarray(adj, np.float32)
    nc = _get_nc()
    in_maps = []
    for c in range(NCORES):
        b, r = c // 4, c % 4
        sl = slice(r * RB, (r + 1) * RB)
        in_maps.append({
            "xb": x[b], "xbr": x[b][sl], "adjr": adj[b][sl],
            "w1": np.asarray(W1, np.float32),
            "wskip1": np.asarray(Wskip1, np.float32),
            "asrc1": np.asarray(a_src1, np.float32),
            "atgt1": np.asarray(a_tgt1, np.float32),
            "b1": np.asarray(b1, np.float32),
            "w2": np.asarray(W2, np.float32),
            "wskip2": np.asarray(Wskip2, np.float32),
            "asrc2": np.asarray(a_src2, np.float32),
            "atgt2": np.asarray(a_tgt2, np.float32),
            "b2": np.asarray(b2, np.float32),
        })
    res = run_bass_kernel_spmd(nc, in_maps, core_ids=list(range(NCORES)))
    out = np.empty((BS, N, F1), np.float32)
    for c in range(NCORES):
        b, r = c // 4, c % 4
        out[b, r * RB:(r + 1) * RB, :] = res.results[c]["outT"].T
    return out
